# revision 27
# baseline (speedup 1.0000x reference)
"""minGRU cell on 8 Trainium2 NeuronCores.

Math (per batch sample, per hidden channel):
    gh    = x @ W.T + b              # (S, 2H): [gate | hidden]
    z_t   = sigmoid(gate_t)
    a_t   = 1 - z_t = sigmoid(-gate_t)
    g_t   = hidden_t + 0.5  if hidden_t >= 0 else sigmoid(hidden_t)
          = relu(hidden_t) + min(sigmoid(hidden_t), 0.5)
    h_t   = a_t * h_{t-1} + z_t * g_t        # linear first-order recurrence

Sharding: data-parallel over batch B=8, one sample per core.

Device layout is channel-major ([channel, time]) everywhere so that
 - the matmul contraction dim (IN) sits on partitions for both operands
   (host pre-transposes x and W — free on host, avoids on-chip transposes),
 - the recurrence runs along the free dim, which is exactly what the DVE
   tensor_tensor_scan instruction implements (fp32 state feedback).
The host transposes the channel-major fp16 result back to (B, S, H) fp32.

The walrus codegen for this toolchain rejects instructions whose sync-wait
list exceeds a (small, per-ISA-struct) budget: Activation, Matmult, the
scan, TensorCopy, and DMA pseudo-instructions all effectively get ONE wait.
Tile emits minimal-but-multiple waits, so this kernel arranges for every
instruction to need at most one:
 - per-engine "guard" instructions absorb cross-engine waits into the
   engine's in-order observed clock (ACT guards observe DVE scan reads,
   a PE ldweights observes the psum-bank releases, DVE guards observe the
   output-store completions),
 - x tiles are never slot-recycled (bufs=NS) so loads carry only their
   DMA-lane wait,
 - outputs are stored in 8 large chunks, one per SWDGE lane, so stores
   never reuse a lane and carry only their scan data wait.
"""

import numpy as np

try:
    import concourse.bass as bass
except ImportError:  # pragma: no cover
    import sys

    sys.path.insert(0, "/opt/trn_rl_repo")
    import concourse.bass as bass

import concourse.mybir as mybir
from concourse.bass_utils import run_bass_kernel_spmd
from concourse.tile import TileContext, add_dep_helper

B, S, IN, H = 8, 8192, 256, 256
N_CORES = 8
SW = 512  # time-tile width (one PSUM bank)
NS = S // SW
G = 4  # s-tiles per output-store group (8 stores total = 8 SWDGE lanes)

_F16 = mybir.dt.float16
_F32 = mybir.dt.float32


def _build():
    nc = bass.Bass()
    Op = mybir.AluOpType
    AF = mybir.ActivationFunctionType

    xT = nc.declare_dram_parameter("xT", [IN, S], _F16, isOutput=False)
    Wt = nc.declare_dram_parameter("Wt", [IN, 2 * H], _F16, isOutput=False)
    bias = nc.declare_dram_parameter("bias", [H, 2], _F32, isOutput=False)
    h0 = nc.declare_dram_parameter("h0", [H, 1], _F32, isOutput=False)
    hT = nc.declare_dram_parameter("hT", [H, S], _F16, isOutput=True)

    with TileContext(nc) as tc:
        with (
            tc.tile_pool(name="const", bufs=1) as cpool,
            tc.tile_pool(name="xin", bufs=NS) as xpool,
            tc.tile_pool(name="work", bufs=3) as wpool,
            tc.tile_pool(name="hout", bufs=2) as hpool,
            tc.tile_pool(name="psum", bufs=2, space="PSUM") as ppool,
        ):
            w_sb = []
            for k in range(2):
                wt = cpool.tile([128, 2 * H], _F16, name=f"w{k}")
                nc.sync.dma_start(out=wt, in_=Wt[k * 128 : (k + 1) * 128, :])
                w_sb.append(wt)
            bias_sb = []
            h0_sb = []
            for cb in range(2):
                bt = cpool.tile([128, 2], _F32, name=f"bias{cb}")
                nc.sync.dma_start(out=bt, in_=bias[cb * 128 : (cb + 1) * 128, :])
                bias_sb.append(bt)
                ht = cpool.tile([128, 1], _F32, name=f"h0{cb}")
                nc.sync.dma_start(out=ht, in_=h0[cb * 128 : (cb + 1) * 128, :])
                h0_sb.append(ht)

            # Route h0 through a DVE copy so the first scans' carry dep is a
            # same-engine dep instead of a DMA wait.
            carry = []
            for cb in range(2):
                c0 = cpool.tile([128, 1], _F32, name=f"carry{cb}")
                nc.vector.tensor_copy(out=c0, in_=h0_sb[cb])
                carry.append(c0[:, 0:1])

            # Make ACT observe the bias DMAs once; real activations then only
            # wait on PE.
            warm0 = cpool.tile([128, 2], _F32, name="warm0")
            warm1 = cpool.tile([128, 2], _F32, name="warm1")
            nc.scalar.copy(out=warm0, in_=bias_sb[0])
            nc.scalar.copy(out=warm1, in_=bias_sb[1])
            # constant f16 source for the DVE store-observe guards
            fc16 = cpool.tile([128, 1], _F16, name="fc16")
            nc.vector.tensor_copy(out=fc16, in_=h0_sb[0])

            WORK_BUFS = 3
            h_slice = [[], []]  # per-cb, per-si scan-output slice APs
            h_group = [None, None]  # per-cb current group tile
            act_hist = []  # per-si psum-reading ACT instructions
            pend_stores = []  # (fire_si, store instruction)
            all_stores = []
            all_loads = []
            last_scans = []
            last_mms = []

            for si in range(NS):
                g, j = divmod(si, G)
                act_hist.append([])

                # PE-engine guard: absorbs the psum-bank release (ACT reads
                # of the bank two tiles ago); must be a real PE-datapath
                # instruction (not a sequencer-only nop).
                pe_guard = None
                if si >= 2:
                    pe_guard = nc.tensor.ldweights(w_sb[0][:, 0:1])
                    for ai in act_hist[si - 2]:
                        add_dep_helper(pe_guard.ins, ai.ins, True, "psum release")

                xk = []
                for k in range(2):
                    xt = xpool.tile([128, SW], _F16, name="xt", tag=f"x{k}")
                    ld = nc.sync.dma_start(
                        out=xt, in_=xT[k * 128 : (k + 1) * 128, si * SW : (si + 1) * SW]
                    )
                    all_loads.append(ld)
                    xk.append(xt)

                # DVE guards: observe output-store completions (two s-tiles of
                # slack); joined into the scans below so slot-recycle WARs
                # need no DMA wait anywhere else.
                dve_guards = []
                while pend_stores and pend_stores[0][0] <= si - 2:
                    _, st_inst = pend_stores.pop(0)
                    dscr = cpool.tile(
                        [128, 1], _F16, name=f"dscr_{si}_{len(dve_guards)}"
                    )
                    gdv = nc.vector.tensor_copy(out=dscr, in_=fc16)
                    add_dep_helper(gdv.ins, st_inst.ins, True, "observe store")
                    dve_guards.append(gdv)

                # ACT guards: ratchet ACT's observed DVE clock past the last
                # DVE reader (the scan) of the work tiles being recycled.
                guards = []
                if si >= WORK_BUFS:
                    for cb in range(2):
                        old_h = h_slice[cb][si - WORK_BUFS]
                        scr = cpool.tile([128, 1], _F16, name=f"scr_{si}_{cb}")
                        gd = nc.scalar.copy(out=scr, in_=old_h[:, 0:1])
                        guards.append(gd)

                for cb in range(2):
                    if j == 0:
                        h_group[cb] = hpool.tile(
                            [128, G * SW], _F16, name="hg", tag=f"hg{cb}"
                        )
                    # gate rows cb*128.., hidden rows H+cb*128.. of gh.T
                    g_ps = ppool.tile([128, SW], _F32, name="g_ps", tag=f"gp{cb}")
                    h_ps = ppool.tile([128, SW], _F32, name="h_ps", tag=f"hp{cb}")
                    for k in range(2):
                        mm = nc.tensor.matmul(
                            g_ps,
                            w_sb[k][:, cb * 128 : (cb + 1) * 128],
                            xk[k],
                            start=(k == 0),
                            stop=(k == 1),
                        )
                        if pe_guard is not None:
                            add_dep_helper(mm.ins, pe_guard.ins, False, "after guard")
                    for k in range(2):
                        mm = nc.tensor.matmul(
                            h_ps,
                            w_sb[k][:, H + cb * 128 : H + (cb + 1) * 128],
                            xk[k],
                            start=(k == 0),
                            stop=(k == 1),
                        )
                        if pe_guard is not None:
                            add_dep_helper(mm.ins, pe_guard.ins, False, "after guard")
                        if si == NS - 1:
                            last_mms.append(mm)
                    # a = sigmoid(-(gate + b_g))   [fp32: scan coefficient]
                    a_sb = wpool.tile([128, SW], _F32, name="a_sb", tag=f"a{cb}")
                    a_inst = nc.scalar.activation(
                        a_sb, g_ps, AF.Sigmoid, bias=bias_sb[cb][:, 0:1], scale=-1.0
                    )
                    for gd in guards:
                        add_dep_helper(a_inst.ins, gd.ins, False, "guard before ACT")
                    act_hist[si].append(a_inst)
                    # zh = sigmoid(hidden + b_h)
                    zh_sb = wpool.tile([128, SW], _F16, name="zh_sb", tag=f"zh{cb}")
                    act_hist[si].append(
                        nc.scalar.activation(
                            zh_sb, h_ps, AF.Sigmoid, bias=bias_sb[cb][:, 1:2], scale=1.0
                        )
                    )
                    # r = relu(hidden + b_h)
                    r_sb = wpool.tile([128, SW], _F16, name="r_sb", tag=f"r{cb}")
                    act_hist[si].append(
                        nc.scalar.activation(
                            r_sb, h_ps, AF.Relu, bias=bias_sb[cb][:, 1:2], scale=1.0
                        )
                    )
                    # z = 1 - a
                    z_sb = wpool.tile([128, SW], _F16, name="z_sb", tag=f"z{cb}")
                    nc.vector.tensor_scalar(z_sb, a_sb, -1.0, 1.0, Op.mult, Op.add)
                    # g = min(zh, 0.5) + r
                    gg_sb = wpool.tile([128, SW], _F16, name="gg_sb", tag=f"gg{cb}")
                    nc.vector.scalar_tensor_tensor(
                        gg_sb, zh_sb, 0.5, r_sb, Op.min, Op.add
                    )
                    # b = z * g
                    b_sb = wpool.tile([128, SW], _F16, name="b_sb", tag=f"b{cb}")
                    nc.vector.tensor_mul(b_sb, z_sb, gg_sb)
                    # h_t = a_t * h_{t-1} + b_t  (fp32 internal state)
                    hh = h_group[cb][:, j * SW : (j + 1) * SW]
                    sc_inst = nc.vector.tensor_tensor_scan(
                        hh, a_sb, b_sb, carry[cb], Op.mult, Op.add
                    )
                    for gdv in dve_guards:
                        add_dep_helper(sc_inst.ins, gdv.ins, True, "join guards")
                    if si == NS - 1:
                        last_scans.append(sc_inst)
                    h_slice[cb].append(hh)
                    carry[cb] = hh[:, SW - 1 : SW]
                    if j == G - 1:
                        st = nc.gpsimd.dma_start(
                            out=hT[
                                cb * 128 : (cb + 1) * 128, g * G * SW : (g + 1) * G * SW
                            ],
                            in_=h_group[cb],
                        )
                        pend_stores.append((si, st))
                        all_stores.append(st)

            # Pre-absorb the kernel-tail drain's waits: a chain of SP nops
            # (each within the control instruction's small wait budget)
            # ratchets SP's observed clock over every outstanding proc, so
            # the Tile-emitted drain needs almost no waits of its own.
            for st in all_stores:
                tn = nc.sync.nop(nofuse=True)
                add_dep_helper(tn.ins, st.ins, True, "tail absorb store")
            tail_deps = all_loads[-8:] + last_scans + last_mms + act_hist[NS - 1]
            for d in tail_deps:
                tn = nc.sync.nop(nofuse=True)
                add_dep_helper(tn.ins, d.ins, True, "tail absorb")
    return nc


_NC_CACHE = None


def _get_nc():
    global _NC_CACHE
    if _NC_CACHE is None:
        _NC_CACHE = _build()
    return _NC_CACHE


def _prepare_in_maps(x, h0, W, b):
    x = np.asarray(x, dtype=np.float32)
    h0 = np.asarray(h0, dtype=np.float32)
    W = np.asarray(W, dtype=np.float32)
    b = np.asarray(b, dtype=np.float32)

    Wt = np.ascontiguousarray(W.T).astype(np.float16)  # [IN, 2H]
    bias_pack = np.ascontiguousarray(
        np.stack([-b[:H], b[H:]], axis=1).astype(np.float32)
    )  # [H, 2]: col0 = -b_gate, col1 = b_hidden

    in_maps = []
    for i in range(N_CORES):
        xTi = x[i].T.astype(np.float16, order="C")  # [IN, S]
        h0i = np.ascontiguousarray(h0[i, 0].reshape(H, 1))  # [H, 1]
        in_maps.append({"xT": xTi, "Wt": Wt, "bias": bias_pack, "h0": h0i})
    return in_maps


def _run(x, h0, W, b, trace=False):
    nc = _get_nc()
    in_maps = _prepare_in_maps(x, h0, W, b)
    res = run_bass_kernel_spmd(nc, in_maps, list(range(N_CORES)), trace=trace)
    out = np.empty((B, S, H), dtype=np.float32)
    for i in range(N_CORES):
        out[i] = res.results[i]["hT"].T.astype(np.float32)
    h_next = out[:, -1:, :].copy()
    return (out, h_next), res


def kernel(x, h0, W, b):
    (out, h_next), _ = _run(x, h0, W, b, trace=False)
    return out, h_next
